# revision 20
# baseline (speedup 1.0000x reference)
"""Causal self-attention (RoPE, 16 heads, D=1024, B=2, T=2048) on 8 TRN2 NeuronCores.

Sharding: tensor-parallel over heads — 2 heads per core. Each core computes the
qkv projection for its heads (fp32r matmuls against host-pre-transposed x),
rotary embedding, causal attention in S^T layout (keys on PSUM partitions so
P^T = exp(S^T) feeds the attn@V matmul directly as the moving operand, with a
ones-column in V producing the softmax denominators on the tensor engine),
and a partial output projection against its slice of out_w rows. The host
sums the 8 partial projections and adds out_b. Work is emitted batch-
interleaved (qkv/rope/attention per batch) so the tensor engine stays dense.
"""

import os

import numpy as np

import concourse.mybir as mybir
import concourse.tile as tile
from concourse import bacc
from concourse.bass_utils import run_bass_kernel_spmd

F32 = mybir.dt.float32
F32R = mybir.dt.float32r
EXP = mybir.ActivationFunctionType.Exp

D = 1024
H = 16
HD = 64
B = 2
T = 2048
BT = B * T            # 4096
NCORES = 8
HLOC = H // NCORES    # 2 heads per core
NDC = D // 128        # 8 contraction chunks for the qkv projection
TBW = 512             # qkv token-block width
NTBB = T // TBW       # 4 token blocks per batch
NKB = T // 128        # 16 key blocks per (b, h)
NQ = T // 512         # 4 query super-blocks per (b, h)
SCALE = float(1.0 / np.sqrt(HD))


def build_nc():
    nc = bacc.Bacc("TRN2", debug=False)

    xT = nc.dram_tensor("xT", [D, BT], F32R, kind="ExternalInput")
    w = nc.dram_tensor("w", [D, 3 * HLOC * HD], F32R, kind="ExternalInput")
    ropeP = nc.dram_tensor("ropeP", [128, T], F32, kind="ExternalInput")
    ropeQ = nc.dram_tensor("ropeQ", [128, T], F32, kind="ExternalInput")
    maskb = nc.dram_tensor("maskb", [128, 640], F32R, kind="ExternalInput")
    ident = nc.dram_tensor("ident", [128, 64], F32, kind="ExternalInput")
    onescol = nc.dram_tensor("onescol", [128, NKB], F32R, kind="ExternalInput")
    wo = nc.dram_tensor("wo", [128, D], F32R, kind="ExternalInput")
    y = nc.dram_tensor("y", [BT, D], F32, kind="ExternalOutput")

    with tile.TileContext(nc) as tc:
        with (
            tc.tile_pool(name="const", bufs=1) as const,
            tc.tile_pool(name="big", bufs=1) as big,
            tc.tile_pool(name="xt", bufs=2) as xt_pool,
            tc.tile_pool(name="raw", bufs=3) as raw_pool,
            tc.tile_pool(name="gsw", bufs=1) as gsw_pool,
            tc.tile_pool(name="rtmp", bufs=2) as rtmp_pool,
            tc.tile_pool(name="p_sb", bufs=3) as p_pool,
            tc.tile_pool(name="r_sb", bufs=2) as r_pool,
            tc.tile_pool(name="rb_sb", bufs=2) as rb_pool,
            tc.tile_pool(name="aTb", bufs=2) as aT_pool,
            tc.tile_pool(name="y_sb", bufs=3) as y_pool,
            tc.tile_pool(name="aux_ps", bufs=2, space="PSUM") as aux_psum,
            tc.tile_pool(name="s_ps", bufs=2, space="PSUM") as s_psum,
            tc.tile_pool(name="o_ps", bufs=2, space="PSUM") as o_psum,
        ):
            w_sb = const.tile([128, NDC, 3 * HLOC * HD], F32R)
            nc.sync.dma_start(w_sb[:], w[:].rearrange("(dc p) f -> p dc f", p=128))
            P_sb = const.tile([128, T], F32)
            nc.sync.dma_start(P_sb[:], ropeP[:])
            Q_sb = const.tile([128, T], F32)
            nc.sync.dma_start(Q_sb[:], ropeQ[:])
            mb_sb = const.tile([128, 640], F32R)
            nc.sync.dma_start(mb_sb[:], maskb[:])
            id_sb = const.tile([128, 64], F32)
            nc.sync.dma_start(id_sb[:], ident[:])
            oc_sb = const.tile([128, NKB], F32R)
            nc.sync.dma_start(oc_sb[:], onescol[:])
            wo_sb = const.tile([128, D], F32R)
            nc.sync.dma_start(wo_sb[:], wo[:])

            qrot = big.tile([128, BT], F32R, tag="qrot")
            krot = big.tile([128, BT], F32R, tag="krot")
            vsb = [
                big.tile([128, NKB, 65], F32R, name=f"v{i}", tag=f"v{i}")
                for i in range(B * HLOC)
            ]

            pending_outproj = []

            def emit_outproj_piece(job, i):
                aTb, row_base = job
                row0 = row_base + i * 128
                ysb = y_pool.tile([128, 2, 512], F32, name=f"ysb{row0}", tag="ysb")
                for nb in range(2):
                    yps = aux_psum.tile(
                        [128, 512], F32, name=f"yps{row0}{nb}", tag="aux"
                    )
                    nc.tensor.matmul(
                        yps[:],
                        aTb[:, i * 128:(i + 1) * 128],
                        wo_sb[:, nb * 512:(nb + 1) * 512],
                        start=True,
                        stop=True,
                    )
                    if nb == 0:
                        nc.vector.tensor_copy(ysb[:, nb, :], yps[:])
                    else:
                        nc.scalar.copy(ysb[:, nb, :], yps[:])
                eng = nc.sync if (row0 // 128) % 2 == 0 else nc.gpsimd
                eng.dma_start(
                    y[row0:row0 + 128, :],
                    ysb[:].rearrange("p a t -> p (a t)"),
                )

            def pop_outproj_piece():
                if pending_outproj:
                    job, i = pending_outproj[0]
                    emit_outproj_piece(job, i)
                    if i == 3:
                        pending_outproj.pop(0)
                    else:
                        pending_outproj[0] = (job, i + 1)

            def phase1_thunks(b):
                """Generate emission thunks for batch b's qkv + rope + V-transpose."""
                state = {}

                def get_raws():
                    if "raws" not in state:
                        state["raws"] = [
                            raw_pool.tile(
                                [128, T], F32R, name=f"raw{b}{ft}", tag="raw"
                            )
                            for ft in range(3)
                        ]
                    return state["raws"]

                def qkv_group(tb, ft):
                    raws = get_raws()
                    if ft == 0:
                        col0 = b * T + tb * TBW
                        xt_t = xt_pool.tile(
                            [128, NDC, TBW], F32R, name=f"xt{b}{tb}", tag="xt"
                        )
                        nc.sync.dma_start(
                            xt_t[:],
                            xT[:, col0:col0 + TBW].rearrange(
                                "(dc p) t -> p dc t", p=128
                            ),
                        )
                        state[("xt", tb)] = xt_t
                    xt_t = state[("xt", tb)]
                    ps = aux_psum.tile(
                        [128, TBW], F32, name=f"qkvps{b}{tb}{ft}", tag="aux"
                    )
                    for dc in range(NDC):
                        nc.tensor.matmul(
                            ps[:],
                            w_sb[:, dc, ft * 128:(ft + 1) * 128],
                            xt_t[:, dc, :],
                            start=(dc == 0),
                            stop=(dc == NDC - 1),
                        )
                    dst = raws[ft][:, tb * TBW:(tb + 1) * TBW]
                    if (tb * 3 + ft) % 2 == 0:
                        nc.scalar.copy(dst, ps[:])
                    else:
                        nc.vector.tensor_copy(dst, ps[:])

                def rope(which):
                    raws = get_raws()
                    raw = raws[0] if which == 0 else raws[1]
                    rot = qrot if which == 0 else krot
                    gsw = gsw_pool.tile([128, T], F32, name=f"gsw{b}{which}", tag="gsw")
                    for l in range(HLOC):
                        p0 = l * 64
                        nc.gpsimd.dma_start(
                            gsw[p0:p0 + 32, :], raw[p0 + 32:p0 + 64, :].bitcast(F32)
                        )
                        nc.gpsimd.dma_start(
                            gsw[p0 + 32:p0 + 64, :], raw[p0:p0 + 32, :].bitcast(F32)
                        )
                    t1 = rtmp_pool.tile([128, T], F32, name=f"rt{b}{which}", tag="rt")
                    nc.vector.tensor_mul(t1[:], raw[:].bitcast(F32), P_sb[:])
                    nc.vector.tensor_mul(gsw[:], gsw[:], Q_sb[:])
                    nc.vector.tensor_add(rot[:, b * T:(b + 1) * T], t1[:], gsw[:])

                def vt_group(l, half):
                    vraw = get_raws()[2]
                    bh = b * HLOC + l
                    if half == 0:
                        nc.gpsimd.dma_start(vsb[bh][:, :, 64], oc_sb[:])
                    for kb in range(half * 8, half * 8 + 8):
                        tp = aux_psum.tile(
                            [128, 64], F32, name=f"tp{b}{l}{kb}", tag="aux"
                        )
                        nc.tensor.transpose(
                            tp[:],
                            vraw[l * 64:(l + 1) * 64,
                                 kb * 128:(kb + 1) * 128].bitcast(F32),
                            id_sb[l * 64:(l + 1) * 64, :],
                        )
                        nc.vector.tensor_copy(vsb[bh][:, kb, 0:64], tp[:])

                thunks = []
                for tb in range(NTBB):
                    for ft in range(3):
                        thunks.append(lambda tb=tb, ft=ft: qkv_group(tb, ft))
                        if tb == NTBB - 1 and ft == 0:
                            thunks.append(lambda: rope(0))
                        if tb == NTBB - 1 and ft == 1:
                            thunks.append(lambda: rope(1))
                for l in range(HLOC):
                    for half in range(2):
                        thunks.append(lambda l=l, half=half: vt_group(l, half))
                return thunks

            def attention(b, filler):
                kb_count = 0
                for qb in range(NQ):
                    q0 = qb * 512
                    nkb = (q0 + 512) // 128
                    opss = [
                        o_psum.tile([65, 512], F32, name=f"ops{b}{qb}{_l}", tag="ops")
                        for _l in range(HLOC)
                    ]

                    def s_pair(kb, _b=b, _qb=qb, _q0=q0):
                        r_off = kb - _qb * 4
                        cm = 128 * r_off if r_off >= 0 else 0
                        k0 = kb * 128
                        ksl = slice(_b * T + k0, _b * T + k0 + 128)
                        qsl = slice(_b * T + _q0 + cm, _b * T + _q0 + 512)
                        sps = s_psum.tile(
                            [128, 2, 512], F32, name=f"sps{_b}{_qb}{kb}", tag="sps",
                        )
                        for l in range(HLOC):
                            p0 = l * 64
                            nc.tensor.matmul(
                                sps[:, l, cm:512],
                                krot[p0:p0 + 64, ksl],
                                qrot[p0:p0 + 64, qsl],
                                start=True,
                                stop=True,
                            )
                        return sps

                    spss = {0: s_pair(0)}
                    for kb in range(nkb):
                        if kb + 1 < nkb:
                            spss[kb + 1] = s_pair(kb + 1)
                        r_off = kb - qb * 4  # >= 0: diagonal-region block
                        pt = p_pool.tile([128, 2, 512], F32R, tag="pt")
                        cm = 128 * r_off if r_off >= 0 else 0
                        sps = spss[kb]
                        if cm == 0:
                            nc.scalar.activation(
                                pt[:].rearrange("p a t -> p (a t)"),
                                sps[:].rearrange("p a t -> p (a t)"),
                                EXP, scale=SCALE,
                            )
                        else:
                            for l in range(HLOC):
                                nc.scalar.activation(
                                    pt[:, l, cm:512], sps[:, l, cm:512],
                                    EXP, scale=SCALE,
                                )
                        if r_off >= 0:
                            for l in range(HLOC):
                                nc.vector.tensor_mul(
                                    pt[:, l, cm:cm + 128],
                                    pt[:, l, cm:cm + 128],
                                    mb_sb[:, 384:512],
                                )
                        for l in range(HLOC):
                            nc.tensor.matmul(
                                opss[l][:, cm:512],
                                vsb[b * HLOC + l][:, kb, :],
                                pt[:, l, cm:512],
                                start=(kb == 0),
                                stop=(kb == nkb - 1),
                            )
                        del spss[kb]
                        pop_outproj_piece()
                        if kb_count % 2 == 1:
                            f = next(filler, None)
                            if f is not None:
                                f()
                        kb_count += 1
                    aTb = aT_pool.tile([128, 512], F32R, name=f"aTb{b}{qb}", tag="aTb")
                    for l in range(HLOC):
                        ops = opss[l]
                        d_sb = r_pool.tile([1, 512], F32, tag="d")
                        nc.vector.tensor_copy(d_sb[:], ops[64:65, :])
                        r_sb = r_pool.tile([1, 512], F32, tag="r")
                        nc.vector.reciprocal_approx_fast(r_sb[:], d_sb[:])
                        rb_sb = rb_pool.tile([64, 512], F32, tag="rb")
                        nc.gpsimd.partition_broadcast(rb_sb[:], r_sb[:])
                        nc.vector.tensor_mul(
                            aTb[l * 64:(l + 1) * 64, :], ops[0:64, :], rb_sb[:]
                        )
                    pending_outproj.append(((aTb, b * T + q0), 0))

            for th in phase1_thunks(0):
                th()
            p1b1 = iter(phase1_thunks(1))
            attention(0, p1b1)
            for th in p1b1:
                th()
            attention(1, iter(()))

            while pending_outproj:
                job, i = pending_outproj.pop(0)
                for j in range(i, 4):
                    emit_outproj_piece(job, j)

    nc.finalize()
    return nc


def _rope_tables():
    inv_freq = 1.0 / (10000.0 ** (np.arange(0, HD, 2, dtype=np.float32) / HD))
    t = np.arange(T, dtype=np.float32)
    freqs = t[:, None] * inv_freq[None, :]                          # [T, 32]
    rope = np.concatenate([np.sin(freqs), np.cos(freqs)], axis=-1)  # [T, 64]
    sin = rope[:, ::2]    # [T, 32]  (reference's "sin")
    cos = rope[:, 1::2]   # [T, 32]  (reference's "cos")
    # rot = raw * P + swap_halves(raw) * Q  with raw rows [x1(32) ; x2(32)]:
    #  rows 0..31  (out half0 = x1*cos - x2*sin; raw=x1, swap=x2): P=cos, Q=-sin
    #  rows 32..63 (out half1 = x1*sin + x2*cos; raw=x2, swap=x1): P=cos, Q=sin
    P64 = np.concatenate([cos.T, cos.T], axis=0)                    # [64, T]
    Q64 = np.concatenate([-sin.T, sin.T], axis=0)                   # [64, T]
    P128 = np.concatenate([P64, P64], axis=0).astype(np.float32)
    Q128 = np.concatenate([Q64, Q64], axis=0).astype(np.float32)
    return np.ascontiguousarray(P128), np.ascontiguousarray(Q128)


def make_core_inputs(x, qkv_w, qkv_b, out_w):
    """Build the per-core input maps for the 8-way head-parallel kernel."""
    x = np.asarray(x, dtype=np.float32)
    qkv_w = np.asarray(qkv_w, dtype=np.float32)
    qkv_b = np.asarray(qkv_b, dtype=np.float32)
    out_w = np.asarray(out_w, dtype=np.float32)
    if np.max(np.abs(qkv_b)) != 0.0:
        raise NotImplementedError("kernel assumes qkv_b == 0 (spec fill: zeros)")

    xT = np.ascontiguousarray(x.reshape(BT, D).T)
    ropeP, ropeQ = _rope_tables()
    deint = np.concatenate([np.arange(0, HD, 2), np.arange(1, HD, 2)])  # [64]
    # maskb[:, 384 - c0 : 512] = [zeros(c0) | tri(128)]; tri valid: col >= row
    maskb = np.concatenate(
        [np.zeros((128, 384), dtype=np.float32),
         np.triu(np.ones((128, 128), dtype=np.float32)),
         np.ones((128, 128), dtype=np.float32)],
        axis=1,
    )
    ident = np.concatenate([np.eye(64, dtype=np.float32)] * 2, axis=0)  # [128, 64]
    onescol = np.ones((128, NKB), dtype=np.float32)

    in_maps = []
    for c in range(NCORES):
        cols = []
        for sect, perm in ((0, deint), (1, deint), (2, np.arange(HD))):
            for l in range(HLOC):
                g = HLOC * c + l
                cols.append(sect * D + g * HD + perm)
        cols = np.concatenate(cols)
        w_core = np.ascontiguousarray(qkv_w[:, cols])
        wo_core = np.ascontiguousarray(out_w[c * 128:(c + 1) * 128, :])
        in_maps.append({
            "xT": xT,
            "w": w_core,
            "ropeP": ropeP,
            "ropeQ": ropeQ,
            "maskb": maskb[:, :640],
            "ident": ident,
            "onescol": onescol,
            "wo": wo_core,
        })
    return in_maps


_NC_CACHE = None


def kernel(x, qkv_w, qkv_b, out_w, out_b):
    global _NC_CACHE
    if _NC_CACHE is None:
        _NC_CACHE = build_nc()
    nc = _NC_CACHE
    in_maps = make_core_inputs(x, qkv_w, qkv_b, out_w)
    trace = bool(os.environ.get("ATTN_KERNEL_TRACE"))
    res = run_bass_kernel_spmd(
        nc, in_maps, core_ids=list(range(NCORES)), trace=trace,
    )
    kernel.last_results = res
    y = res.results[0]["y"].astype(np.float64)
    for c in range(1, NCORES):
        y = y + res.results[c]["y"].astype(np.float64)
    y = y + np.asarray(out_b, dtype=np.float64)[None, :]
    return np.ascontiguousarray(y.reshape(B, T, D).astype(np.float32))


# revision 21
# speedup vs baseline: 1.0332x; 1.0332x over previous
"""Causal self-attention (RoPE, 16 heads, D=1024, B=2, T=2048) on 8 TRN2 NeuronCores.

Sharding: tensor-parallel over heads — 2 heads per core. Each core computes the
qkv projection for its heads (fp32r matmuls against host-pre-transposed x),
rotary embedding, causal attention in S^T layout (keys on PSUM partitions so
P^T = exp(S^T) feeds the attn@V matmul directly as the moving operand, with a
ones-column in V producing the softmax denominators on the tensor engine),
and a partial output projection against its slice of out_w rows. The host
sums the 8 partial projections and adds out_b. Work is emitted batch-
interleaved (qkv/rope/attention per batch) so the tensor engine stays dense.
"""

import os

import numpy as np

import concourse.mybir as mybir
import concourse.tile as tile
from concourse import bacc
from concourse.bass_utils import run_bass_kernel_spmd

F32 = mybir.dt.float32
F32R = mybir.dt.float32r
EXP = mybir.ActivationFunctionType.Exp

D = 1024
H = 16
HD = 64
B = 2
T = 2048
BT = B * T            # 4096
NCORES = 8
HLOC = H // NCORES    # 2 heads per core
NDC = D // 128        # 8 contraction chunks for the qkv projection
TBW = 512             # qkv token-block width
NTBB = T // TBW       # 4 token blocks per batch
NKB = T // 128        # 16 key blocks per (b, h)
NQ = T // 512         # 4 query super-blocks per (b, h)
SCALE = float(1.0 / np.sqrt(HD))


def build_nc():
    nc = bacc.Bacc("TRN2", debug=False)

    xT = nc.dram_tensor("xT", [D, BT], F32R, kind="ExternalInput")
    w = nc.dram_tensor("w", [D, 3 * HLOC * HD], F32R, kind="ExternalInput")
    ropeP = nc.dram_tensor("ropeP", [128, T], F32, kind="ExternalInput")
    ropeQ = nc.dram_tensor("ropeQ", [128, T], F32, kind="ExternalInput")
    maskb = nc.dram_tensor("maskb", [128, 640], F32R, kind="ExternalInput")
    ident = nc.dram_tensor("ident", [128, 64], F32, kind="ExternalInput")
    onescol = nc.dram_tensor("onescol", [128, NKB], F32R, kind="ExternalInput")
    wo = nc.dram_tensor("wo", [128, D], F32R, kind="ExternalInput")
    y = nc.dram_tensor("y", [BT, D], F32, kind="ExternalOutput")

    with tile.TileContext(nc) as tc:
        with (
            tc.tile_pool(name="const", bufs=1) as const,
            tc.tile_pool(name="big", bufs=1) as big,
            tc.tile_pool(name="xt", bufs=2) as xt_pool,
            tc.tile_pool(name="raw", bufs=3) as raw_pool,
            tc.tile_pool(name="gsw", bufs=1) as gsw_pool,
            tc.tile_pool(name="rtmp", bufs=2) as rtmp_pool,
            tc.tile_pool(name="p_sb", bufs=3) as p_pool,
            tc.tile_pool(name="r_sb", bufs=2) as r_pool,
            tc.tile_pool(name="rb_sb", bufs=2) as rb_pool,
            tc.tile_pool(name="aTb", bufs=2) as aT_pool,
            tc.tile_pool(name="y_sb", bufs=3) as y_pool,
            tc.tile_pool(name="aux_ps", bufs=2, space="PSUM") as aux_psum,
            tc.tile_pool(name="s_ps", bufs=2, space="PSUM") as s_psum,
            tc.tile_pool(name="o_ps", bufs=2, space="PSUM") as o_psum,
        ):
            w_sb = const.tile([128, NDC, 3 * HLOC * HD], F32R)
            nc.sync.dma_start(w_sb[:], w[:].rearrange("(dc p) f -> p dc f", p=128))
            P_sb = const.tile([128, T], F32)
            nc.sync.dma_start(P_sb[:], ropeP[:])
            Q_sb = const.tile([128, T], F32)
            nc.sync.dma_start(Q_sb[:], ropeQ[:])
            mb_sb = const.tile([128, 640], F32R)
            nc.sync.dma_start(mb_sb[:], maskb[:])
            id_sb = const.tile([128, 64], F32)
            nc.sync.dma_start(id_sb[:], ident[:])
            oc_sb = const.tile([128, NKB], F32R)
            nc.sync.dma_start(oc_sb[:], onescol[:])
            wo_sb = const.tile([128, D], F32R)
            nc.sync.dma_start(wo_sb[:], wo[:])

            qrot = big.tile([128, BT], F32R, tag="qrot")
            krot = big.tile([128, BT], F32R, tag="krot")
            vsb = [
                big.tile([128, NKB, 65], F32R, name=f"v{i}", tag=f"v{i}")
                for i in range(B * HLOC)
            ]

            pending_outproj = []

            def emit_outproj_piece(job, i):
                aTb, row_base = job
                row0 = row_base + i * 128
                ysb = y_pool.tile([128, 2, 512], F32, name=f"ysb{row0}", tag="ysb")
                for nb in range(2):
                    yps = aux_psum.tile(
                        [128, 512], F32, name=f"yps{row0}{nb}", tag="aux"
                    )
                    nc.tensor.matmul(
                        yps[:],
                        aTb[:, i * 128:(i + 1) * 128],
                        wo_sb[:, nb * 512:(nb + 1) * 512],
                        start=True,
                        stop=True,
                    )
                    if nb == 0:
                        nc.vector.tensor_copy(ysb[:, nb, :], yps[:])
                    else:
                        nc.scalar.copy(ysb[:, nb, :], yps[:])
                nc.sync.dma_start(
                    y[row0:row0 + 128, :],
                    ysb[:].rearrange("p a t -> p (a t)"),
                )

            def pop_outproj_piece():
                if pending_outproj:
                    job, i = pending_outproj[0]
                    emit_outproj_piece(job, i)
                    if i == 3:
                        pending_outproj.pop(0)
                    else:
                        pending_outproj[0] = (job, i + 1)

            def phase1_thunks(b):
                """Generate emission thunks for batch b's qkv + rope + V-transpose."""
                state = {}

                def get_raws():
                    if "raws" not in state:
                        state["raws"] = [
                            raw_pool.tile(
                                [128, T], F32R, name=f"raw{b}{ft}", tag="raw"
                            )
                            for ft in range(3)
                        ]
                    return state["raws"]

                def qkv_group(tb, ft):
                    raws = get_raws()
                    if ft == 0:
                        col0 = b * T + tb * TBW
                        xt_t = xt_pool.tile(
                            [128, NDC, TBW], F32R, name=f"xt{b}{tb}", tag="xt"
                        )
                        nc.sync.dma_start(
                            xt_t[:],
                            xT[:, col0:col0 + TBW].rearrange(
                                "(dc p) t -> p dc t", p=128
                            ),
                        )
                        state[("xt", tb)] = xt_t
                    xt_t = state[("xt", tb)]
                    ps = aux_psum.tile(
                        [128, TBW], F32, name=f"qkvps{b}{tb}{ft}", tag="aux"
                    )
                    for dc in range(NDC):
                        nc.tensor.matmul(
                            ps[:],
                            w_sb[:, dc, ft * 128:(ft + 1) * 128],
                            xt_t[:, dc, :],
                            start=(dc == 0),
                            stop=(dc == NDC - 1),
                        )
                    dst = raws[ft][:, tb * TBW:(tb + 1) * TBW]
                    nc.scalar.copy(dst, ps[:])

                def rope(which):
                    raws = get_raws()
                    raw = raws[0] if which == 0 else raws[1]
                    rot = qrot if which == 0 else krot
                    gsw = gsw_pool.tile([128, T], F32, name=f"gsw{b}{which}", tag="gsw")
                    for l in range(HLOC):
                        p0 = l * 64
                        nc.gpsimd.dma_start(
                            gsw[p0:p0 + 32, :], raw[p0 + 32:p0 + 64, :].bitcast(F32)
                        )
                        nc.gpsimd.dma_start(
                            gsw[p0 + 32:p0 + 64, :], raw[p0:p0 + 32, :].bitcast(F32)
                        )
                    t1 = rtmp_pool.tile([128, T], F32, name=f"rt{b}{which}", tag="rt")
                    nc.vector.tensor_mul(t1[:], raw[:].bitcast(F32), P_sb[:])
                    nc.vector.tensor_mul(gsw[:], gsw[:], Q_sb[:])
                    nc.vector.tensor_add(rot[:, b * T:(b + 1) * T], t1[:], gsw[:])

                def vt_group(l, half):
                    vraw = get_raws()[2]
                    bh = b * HLOC + l
                    if half == 0:
                        nc.gpsimd.dma_start(vsb[bh][:, :, 64], oc_sb[:])
                    for kb in range(half * 8, half * 8 + 8):
                        tp = aux_psum.tile(
                            [128, 64], F32, name=f"tp{b}{l}{kb}", tag="aux"
                        )
                        nc.tensor.transpose(
                            tp[:],
                            vraw[l * 64:(l + 1) * 64,
                                 kb * 128:(kb + 1) * 128].bitcast(F32),
                            id_sb[l * 64:(l + 1) * 64, :],
                        )
                        nc.vector.tensor_copy(vsb[bh][:, kb, 0:64], tp[:])

                thunks = []
                for tb in range(NTBB):
                    for ft in range(3):
                        thunks.append(lambda tb=tb, ft=ft: qkv_group(tb, ft))
                        if tb == NTBB - 1 and ft == 0:
                            thunks.append(lambda: rope(0))
                        if tb == NTBB - 1 and ft == 1:
                            thunks.append(lambda: rope(1))
                for l in range(HLOC):
                    for half in range(2):
                        thunks.append(lambda l=l, half=half: vt_group(l, half))
                return thunks

            def attention(b, filler):
                kb_count = 0
                for qb in range(NQ):
                    q0 = qb * 512
                    nkb = (q0 + 512) // 128
                    opss = [
                        o_psum.tile([65, 512], F32, name=f"ops{b}{qb}{_l}", tag="ops")
                        for _l in range(HLOC)
                    ]

                    def s_pair(kb, _b=b, _qb=qb, _q0=q0):
                        r_off = kb - _qb * 4
                        cm = 128 * r_off if r_off >= 0 else 0
                        k0 = kb * 128
                        ksl = slice(_b * T + k0, _b * T + k0 + 128)
                        qsl = slice(_b * T + _q0 + cm, _b * T + _q0 + 512)
                        sps = s_psum.tile(
                            [128, 2, 512], F32, name=f"sps{_b}{_qb}{kb}", tag="sps",
                        )
                        for l in range(HLOC):
                            p0 = l * 64
                            nc.tensor.matmul(
                                sps[:, l, cm:512],
                                krot[p0:p0 + 64, ksl],
                                qrot[p0:p0 + 64, qsl],
                                start=True,
                                stop=True,
                            )
                        return sps

                    spss = {0: s_pair(0)}
                    for kb in range(nkb):
                        if kb + 1 < nkb:
                            spss[kb + 1] = s_pair(kb + 1)
                        r_off = kb - qb * 4  # >= 0: diagonal-region block
                        pt = p_pool.tile([128, 2, 512], F32R, tag="pt")
                        cm = 128 * r_off if r_off >= 0 else 0
                        sps = spss[kb]
                        if cm == 0:
                            nc.scalar.activation(
                                pt[:].rearrange("p a t -> p (a t)"),
                                sps[:].rearrange("p a t -> p (a t)"),
                                EXP, scale=SCALE,
                            )
                        else:
                            for l in range(HLOC):
                                nc.scalar.activation(
                                    pt[:, l, cm:512], sps[:, l, cm:512],
                                    EXP, scale=SCALE,
                                )
                        if r_off >= 0:
                            for l in range(HLOC):
                                nc.vector.tensor_mul(
                                    pt[:, l, cm:cm + 128],
                                    pt[:, l, cm:cm + 128],
                                    mb_sb[:, 384:512],
                                )
                        for l in range(HLOC):
                            nc.tensor.matmul(
                                opss[l][:, cm:512],
                                vsb[b * HLOC + l][:, kb, :],
                                pt[:, l, cm:512],
                                start=(kb == 0),
                                stop=(kb == nkb - 1),
                            )
                        del spss[kb]
                        pop_outproj_piece()
                        if kb_count % 2 == 1:
                            f = next(filler, None)
                            if f is not None:
                                f()
                        kb_count += 1
                    aTb = aT_pool.tile([128, 512], F32R, name=f"aTb{b}{qb}", tag="aTb")
                    for l in range(HLOC):
                        ops = opss[l]
                        d_sb = r_pool.tile([1, 512], F32, tag="d")
                        nc.vector.tensor_copy(d_sb[:], ops[64:65, :])
                        r_sb = r_pool.tile([1, 512], F32, tag="r")
                        nc.vector.reciprocal_approx_fast(r_sb[:], d_sb[:])
                        rb_sb = rb_pool.tile([64, 512], F32, tag="rb")
                        nc.gpsimd.partition_broadcast(rb_sb[:], r_sb[:])
                        nc.vector.tensor_mul(
                            aTb[l * 64:(l + 1) * 64, :], ops[0:64, :], rb_sb[:]
                        )
                    pending_outproj.append(((aTb, b * T + q0), 0))

            for th in phase1_thunks(0):
                th()
            p1b1 = iter(phase1_thunks(1))
            attention(0, p1b1)
            for th in p1b1:
                th()
            attention(1, iter(()))

            while pending_outproj:
                job, i = pending_outproj.pop(0)
                for j in range(i, 4):
                    emit_outproj_piece(job, j)

    nc.finalize()
    return nc


def _rope_tables():
    inv_freq = 1.0 / (10000.0 ** (np.arange(0, HD, 2, dtype=np.float32) / HD))
    t = np.arange(T, dtype=np.float32)
    freqs = t[:, None] * inv_freq[None, :]                          # [T, 32]
    rope = np.concatenate([np.sin(freqs), np.cos(freqs)], axis=-1)  # [T, 64]
    sin = rope[:, ::2]    # [T, 32]  (reference's "sin")
    cos = rope[:, 1::2]   # [T, 32]  (reference's "cos")
    # rot = raw * P + swap_halves(raw) * Q  with raw rows [x1(32) ; x2(32)]:
    #  rows 0..31  (out half0 = x1*cos - x2*sin; raw=x1, swap=x2): P=cos, Q=-sin
    #  rows 32..63 (out half1 = x1*sin + x2*cos; raw=x2, swap=x1): P=cos, Q=sin
    P64 = np.concatenate([cos.T, cos.T], axis=0)                    # [64, T]
    Q64 = np.concatenate([-sin.T, sin.T], axis=0)                   # [64, T]
    P128 = np.concatenate([P64, P64], axis=0).astype(np.float32)
    Q128 = np.concatenate([Q64, Q64], axis=0).astype(np.float32)
    return np.ascontiguousarray(P128), np.ascontiguousarray(Q128)


def make_core_inputs(x, qkv_w, qkv_b, out_w):
    """Build the per-core input maps for the 8-way head-parallel kernel."""
    x = np.asarray(x, dtype=np.float32)
    qkv_w = np.asarray(qkv_w, dtype=np.float32)
    qkv_b = np.asarray(qkv_b, dtype=np.float32)
    out_w = np.asarray(out_w, dtype=np.float32)
    if np.max(np.abs(qkv_b)) != 0.0:
        raise NotImplementedError("kernel assumes qkv_b == 0 (spec fill: zeros)")

    xT = np.ascontiguousarray(x.reshape(BT, D).T)
    ropeP, ropeQ = _rope_tables()
    deint = np.concatenate([np.arange(0, HD, 2), np.arange(1, HD, 2)])  # [64]
    # maskb[:, 384 - c0 : 512] = [zeros(c0) | tri(128)]; tri valid: col >= row
    maskb = np.concatenate(
        [np.zeros((128, 384), dtype=np.float32),
         np.triu(np.ones((128, 128), dtype=np.float32)),
         np.ones((128, 128), dtype=np.float32)],
        axis=1,
    )
    ident = np.concatenate([np.eye(64, dtype=np.float32)] * 2, axis=0)  # [128, 64]
    onescol = np.ones((128, NKB), dtype=np.float32)

    in_maps = []
    for c in range(NCORES):
        cols = []
        for sect, perm in ((0, deint), (1, deint), (2, np.arange(HD))):
            for l in range(HLOC):
                g = HLOC * c + l
                cols.append(sect * D + g * HD + perm)
        cols = np.concatenate(cols)
        w_core = np.ascontiguousarray(qkv_w[:, cols])
        wo_core = np.ascontiguousarray(out_w[c * 128:(c + 1) * 128, :])
        in_maps.append({
            "xT": xT,
            "w": w_core,
            "ropeP": ropeP,
            "ropeQ": ropeQ,
            "maskb": maskb[:, :640],
            "ident": ident,
            "onescol": onescol,
            "wo": wo_core,
        })
    return in_maps


_NC_CACHE = None


def kernel(x, qkv_w, qkv_b, out_w, out_b):
    global _NC_CACHE
    if _NC_CACHE is None:
        _NC_CACHE = build_nc()
    nc = _NC_CACHE
    in_maps = make_core_inputs(x, qkv_w, qkv_b, out_w)
    trace = bool(os.environ.get("ATTN_KERNEL_TRACE"))
    res = run_bass_kernel_spmd(
        nc, in_maps, core_ids=list(range(NCORES)), trace=trace,
    )
    kernel.last_results = res
    y = res.results[0]["y"].astype(np.float64)
    for c in range(1, NCORES):
        y = y + res.results[c]["y"].astype(np.float64)
    y = y + np.asarray(out_b, dtype=np.float64)[None, :]
    return np.ascontiguousarray(y.reshape(B, T, D).astype(np.float32))
